# revision 1
# baseline (speedup 1.0000x reference)
"""Trainium2 Bass kernel for phase-field fracture FEM energy (gnn_message_passing).

Sharding: elements split across 8 NeuronCores (data-parallel); nodal arrays
enter element space via a uvc gather done during host-side input prep; the
three scalar energy sums are reduced per-(core, partition) on device and the
final reduction happens on host (the output-unshard step).

Device kernel per core (32768 elements = 128 partitions x 256 elems):
  - streams N/dNdx/B (bf16), uvc_el/volumes (f32)
  - einsums over nodes-per-element via tensor_tensor mult + free-axis reduce
  - fracture + Amor-split elastic energy densities, fused scale constants
  - E_irr from a node shard of (c, prev_c)
  - accumulates 3 partial sums per partition -> out [128, 4]
"""
import numpy as np
import ml_dtypes

# --- problem constants (from reference) --------------------------------------
G_C = 0.0027
L_0 = 0.015
PF_TOL = 0.01
ENERGY_SCALING = 1.0
NU = 0.3
E_MOD = 210.0
LAM = E_MOD * NU / ((1.0 + NU) * (1.0 - 2.0 * NU))
MU = E_MOD / (2.0 * (1.0 + NU))
K_MOD = LAM + 2.0 * MU / 3.0
PENALTY = G_C / L_0 * (1.0 / PF_TOL**2 - 1.0) * ENERGY_SCALING

N_NODES = 263169
N_ELEMS = 262144
NCORES = 8
P = 128
EC = N_ELEMS // NCORES          # 32768 elements per core
EPP = EC // P                   # 256 elements per partition
TE = 64                         # elements per partition per tile (tunable)
NT = EPP // TE                  # tiles
NODE_PAD = 33024                # per-core node shard (128*258), 8*33024 >= N_NODES
NODE_F = NODE_PAD // P          # 258

TRACE = False
SIM_EXEC_NS = 56247  # CoreSim cost-model predicted per-core exec (no NTFF profiling under axon)
COMPUTE = True
LOAD_BUFS = 2
SCRATCH_BUFS = 2
LAST_EXEC_NS = None  # populated only when NTFF tracing is available
_CACHE = {}


def _build_bass():
    import concourse.bacc as bacc
    import concourse.tile as tile
    from concourse import mybir

    f32 = mybir.dt.float32
    bf16 = mybir.dt.bfloat16
    Alu = mybir.AluOpType
    Act = mybir.ActivationFunctionType

    TE_ = TE
    NT_ = EPP // TE_
    nc = bacc.Bacc("TRN2")
    d_uvc = nc.dram_tensor("uvc", [P, EPP * 12], f32, kind="ExternalInput")
    d_n = nc.dram_tensor("nmat", [P, EPP * 16], bf16, kind="ExternalInput")
    d_dn = nc.dram_tensor("dmat", [P, EPP * 32], bf16, kind="ExternalInput")
    d_b = nc.dram_tensor("bmat", [P, EPP * 96], bf16, kind="ExternalInput")
    d_vol = nc.dram_tensor("vol", [P, EPP * 4], f32, kind="ExternalInput")
    d_c = nc.dram_tensor("cnd", [P, NODE_F], f32, kind="ExternalInput")
    d_pc = nc.dram_tensor("pnd", [P, NODE_F], f32, kind="ExternalInput")
    d_out = nc.dram_tensor("out", [P, 4], f32, kind="ExternalOutput")

    with tile.TileContext(nc) as tc:
        with (
            tc.tile_pool(name="loads", bufs=LOAD_BUFS) as loads,
            tc.tile_pool(name="scratch", bufs=SCRATCH_BUFS) as scratch,
            tc.tile_pool(name="acc", bufs=1) as accp,
        ):
            accE = accp.tile([P, 1], f32)
            accF = accp.tile([P, 1], f32)
            accI = accp.tile([P, 1], f32)
            nc.vector.memset(accE[:], 0.0)
            nc.vector.memset(accF[:], 0.0)
            nc.vector.memset(accI[:], 0.0)

            # ---- E_irr over the node shard ---------------------------------
            t_c = accp.tile([P, NODE_F], f32)
            t_pc = accp.tile([P, NODE_F], f32)
            nc.sync.dma_start(out=t_c[:], in_=d_c[:])
            nc.sync.dma_start(out=t_pc[:], in_=d_pc[:])
            t_d = accp.tile([P, NODE_F], f32)
            nc.vector.tensor_tensor(out=t_d[:], in0=t_pc[:], in1=t_c[:], op=Alu.subtract)
            t_r = accp.tile([P, NODE_F], f32)
            nc.scalar.activation(out=t_r[:], in_=t_d[:], func=Act.Relu, bias=0.0, scale=1.0)
            t_sc = accp.tile([P, NODE_F], f32)
            nc.vector.tensor_tensor(out=t_sc[:], in0=t_r[:], in1=t_r[:], op=Alu.mult)
            t_ired = accp.tile([P, 1], f32)
            nc.vector.tensor_reduce(out=t_ired[:], in_=t_sc[:], axis=mybir.AxisListType.X, op=Alu.add)
            nc.vector.tensor_tensor(out=accI[:], in0=accI[:], in1=t_ired[:], op=Alu.add)

            # ---- element tiles ---------------------------------------------
            sizes = [16, 48] + [TE_] * ((EPP - 64) // TE_)
            assert sum(sizes) == EPP
            offs = [sum(sizes[:i]) for i in range(len(sizes))]
            for t, (eo, sz) in enumerate(zip(offs, sizes)):
                sl12 = slice(eo * 12, (eo + sz) * 12)
                sl16 = slice(eo * 16, (eo + sz) * 16)
                sl32 = slice(eo * 32, (eo + sz) * 32)
                sl96 = slice(eo * 96, (eo + sz) * 96)
                sl4 = slice(eo * 4, (eo + sz) * 4)

                t_uvc = loads.tile([P, sz * 12], f32)
                t_n = loads.tile([P, sz * 16], bf16)
                t_dn = loads.tile([P, sz * 32], bf16)
                t_b = loads.tile([P, sz * 96], bf16)
                t_vol = loads.tile([P, sz * 4], f32)
                nc.sync.dma_start(out=t_uvc[:], in_=d_uvc[:, sl12])
                nc.sync.dma_start(out=t_n[:], in_=d_n[:, sl16])
                nc.sync.dma_start(out=t_dn[:], in_=d_dn[:, sl32])
                nc.sync.dma_start(out=t_b[:], in_=d_b[:, sl96])
                nc.sync.dma_start(out=t_vol[:], in_=d_vol[:, sl4])

                if not COMPUTE:
                    continue
                uvc_r = t_uvc[:].rearrange("p (e n c) -> p e n c", n=4, c=3)
                n_r = t_n[:].rearrange("p (e i n) -> p e i n", i=4, n=4)
                dn_r = t_dn[:].rearrange("p (e x n) -> p e x n", x=8, n=4)
                b_r = t_b[:].rearrange("p (e y j) -> p e y j", y=12, j=8)
                vol_f = t_vol[:]  # [P, TE*4]

                # contiguous bf16 copy of c_el; broadcast views from it
                t_cel = scratch.tile([P, sz, 4], bf16)
                nc.scalar.copy(out=t_cel[:], in_=uvc_r[:, :, :, 2:3].squeeze(3))
                cel_ip = t_cel[:].unsqueeze(2).broadcast_to([P, sz, 4, 4])
                cel_g = t_cel[:].unsqueeze(2).broadcast_to([P, sz, 8, 4])

                # uv interleave [p e 8] <- uvc[..., 0:2]
                t_uv = scratch.tile([P, sz * 8], bf16)
                uv_w = t_uv[:].rearrange("p (e n c) -> p e n c", n=4, c=2)
                nc.scalar.copy(out=uv_w, in_=uvc_r[:, :, :, 0:2])
                uv_b = (
                    t_uv[:].rearrange("p (e j) -> p e j", j=8)
                    .unsqueeze(2).broadcast_to([P, sz, 12, 8])
                )

                # nc_ip = sum_n N * c_el  -> [p e i]
                t_m1 = scratch.tile([P, sz, 4, 4], bf16)
                nc.gpsimd.tensor_tensor(out=t_m1[:], in0=n_r, in1=cel_ip, op=Alu.mult)
                t_m1h = scratch.tile([P, sz, 4, 2], f32)
                nc.gpsimd.tensor_tensor(out=t_m1h[:], in0=t_m1[:, :, :, 0:2], in1=t_m1[:, :, :, 2:4], op=Alu.add)
                t_nc = scratch.tile([P, sz, 4], f32)
                nc.vector.tensor_tensor(out=t_nc[:], in0=t_m1h[:, :, :, 0:1].squeeze(3), in1=t_m1h[:, :, :, 1:2].squeeze(3), op=Alu.add)

                # grad = sum_n dNdx * c_el -> [p e i d]
                t_m2 = scratch.tile([P, sz, 8, 4], bf16)
                nc.gpsimd.tensor_tensor(out=t_m2[:], in0=dn_r, in1=cel_g, op=Alu.mult)
                t_m2h = scratch.tile([P, sz, 8, 2], f32)
                nc.gpsimd.tensor_tensor(out=t_m2h[:], in0=t_m2[:, :, :, 0:2], in1=t_m2[:, :, :, 2:4], op=Alu.add)
                t_gr = scratch.tile([P, sz, 4, 2], f32)
                nc.vector.tensor_tensor(out=t_gr[:].rearrange("p e i d -> p (e i d)").rearrange("p (x) -> p x"), in0=t_m2h[:, :, :, 0:1].squeeze(3).rearrange("p e x -> p (e x)"), in1=t_m2h[:, :, :, 1:2].squeeze(3).rearrange("p e x -> p (e x)"), op=Alu.add)

                # gsq = grad_x^2 + grad_y^2 -> [p e i]
                t_g2 = scratch.tile([P, sz, 4, 2], f32)
                nc.gpsimd.tensor_tensor(out=t_g2[:], in0=t_gr[:], in1=t_gr[:], op=Alu.mult)
                t_gs = scratch.tile([P, sz, 4], f32)
                nc.vector.tensor_reduce(out=t_gs[:], in_=t_g2[:], axis=mybir.AxisListType.X, op=Alu.add)

                # q = nc^2 + L0^2 * gsq ; E_frac partial += q * vol
                t_cs = scratch.tile([P, sz, 4], f32)
                nc.scalar.activation(out=t_cs[:], in_=t_nc[:], func=Act.Square, bias=0.0, scale=1.0)
                t_q = scratch.tile([P, sz, 4], f32)
                nc.scalar.activation(out=t_q[:], in_=t_gs[:], func=Act.Copy, bias=0.0, scale=float(L_0 * L_0))
                t_qt = scratch.tile([P, sz, 4], f32)
                nc.vector.tensor_tensor(out=t_qt[:], in0=t_q[:], in1=t_cs[:], op=Alu.add)
                t_s1 = scratch.tile([P, sz * 4], f32)
                nc.vector.tensor_tensor(out=t_s1[:], in0=t_qt[:].rearrange("p e i -> p (e i)"), in1=vol_f, op=Alu.mult)
                t_fred = scratch.tile([P, 1], f32)
                nc.vector.tensor_reduce(out=t_fred[:], in_=t_s1[:], axis=mybir.AxisListType.X, op=Alu.add)
                nc.vector.tensor_tensor(out=accF[:], in0=accF[:], in1=t_fred[:], op=Alu.add)

                # strain = sum_j B * uv -> [p e i k]
                t_m4 = scratch.tile([P, sz, 12, 8], bf16)
                nc.vector.tensor_tensor(out=t_m4[:, :, 0:8, :], in0=b_r[:, :, 0:8, :], in1=uv_b[:, :, 0:8, :], op=Alu.mult)
                nc.gpsimd.tensor_tensor(out=t_m4[:, :, 8:12, :], in0=b_r[:, :, 8:12, :], in1=uv_b[:, :, 8:12, :], op=Alu.mult)
                t_m4h = scratch.tile([P, sz, 12, 4], f32)
                nc.gpsimd.tensor_tensor(out=t_m4h[:], in0=t_m4[:, :, :, 0:4], in1=t_m4[:, :, :, 4:8], op=Alu.add)
                t_st = scratch.tile([P, sz, 4, 3], f32)
                nc.vector.tensor_reduce(out=t_st[:], in_=t_m4h[:].rearrange("p e y j -> p (e y) j"), axis=mybir.AxisListType.X, op=Alu.add)

                a_v = t_st[:, :, :, 0:1].squeeze(3)
                b_v = t_st[:, :, :, 1:2].squeeze(3)
                s2_v = t_st[:, :, :, 2:3].squeeze(3)

                # tr = a+b ; sd = a-b ; dev2 = tr^2/6 + sd^2/2 + s2^2/2
                t_tr = scratch.tile([P, sz, 4], f32)
                nc.vector.tensor_tensor(out=t_tr[:], in0=a_v, in1=b_v, op=Alu.add)
                t_sd = scratch.tile([P, sz, 4], f32)
                nc.vector.tensor_tensor(out=t_sd[:], in0=a_v, in1=b_v, op=Alu.subtract)
                t_rp = scratch.tile([P, sz, 4], f32)
                nc.scalar.activation(out=t_rp[:], in_=t_tr[:], func=Act.Relu, bias=0.0, scale=1.0)
                t_rps = scratch.tile([P, sz, 4], f32)
                nc.scalar.activation(out=t_rps[:], in_=t_rp[:], func=Act.Square, bias=0.0, scale=float((0.5 * K_MOD) ** 0.5))
                t_rns = scratch.tile([P, sz, 4], f32)
                nc.scalar.activation(out=t_rns[:], in_=t_tr[:], func=Act.Relu, bias=0.0, scale=-1.0)
                t_rnsq = scratch.tile([P, sz, 4], f32)
                nc.scalar.activation(out=t_rnsq[:], in_=t_rns[:], func=Act.Square, bias=0.0, scale=float((0.5 * K_MOD) ** 0.5))
                t_trs = scratch.tile([P, sz, 4], f32)
                nc.scalar.activation(out=t_trs[:], in_=t_tr[:], func=Act.Square, bias=0.0, scale=float((MU / 6.0) ** 0.5))
                t_sds = scratch.tile([P, sz, 4], f32)
                nc.scalar.activation(out=t_sds[:], in_=t_sd[:], func=Act.Square, bias=0.0, scale=float((0.5 * MU) ** 0.5))
                t_ss = scratch.tile([P, sz, 4], f32)
                nc.scalar.activation(out=t_ss[:], in_=s2_v, func=Act.Square, bias=0.0, scale=float((0.5 * MU) ** 0.5))

                # m = 1 - nc ; g = m^2
                t_mm = scratch.tile([P, sz, 4], f32)
                nc.scalar.activation(out=t_mm[:], in_=t_nc[:], func=Act.Copy, bias=1.0, scale=-1.0)
                t_gg = scratch.tile([P, sz, 4], f32)
                nc.scalar.activation(out=t_gg[:], in_=t_mm[:], func=Act.Square, bias=0.0, scale=1.0)

                # zp = 0.5K*rps + MU/6*trs + MU/2*sds + MU/2*ss ; psim = 0.5K*rnsq
                t_z12 = scratch.tile([P, sz, 4], f32)
                nc.gpsimd.tensor_tensor(out=t_z12[:], in0=t_rps[:], in1=t_trs[:], op=Alu.add)
                t_z34 = scratch.tile([P, sz, 4], f32)
                nc.gpsimd.tensor_tensor(out=t_z34[:], in0=t_sds[:], in1=t_ss[:], op=Alu.add)
                t_zp = scratch.tile([P, sz, 4], f32)
                nc.gpsimd.tensor_tensor(out=t_zp[:], in0=t_z12[:], in1=t_z34[:], op=Alu.add)
                t_zg = scratch.tile([P, sz, 4], f32)
                nc.vector.tensor_tensor(out=t_zg[:], in0=t_zp[:], in1=t_gg[:], op=Alu.mult)
                t_cb = scratch.tile([P, sz, 4], f32)
                nc.vector.tensor_tensor(out=t_cb[:], in0=t_zg[:], in1=t_rnsq[:], op=Alu.add)
                t_s2c = scratch.tile([P, sz * 4], f32)
                nc.vector.tensor_tensor(out=t_s2c[:], in0=t_cb[:].rearrange("p e i -> p (e i)"), in1=vol_f, op=Alu.mult)
                t_ered = scratch.tile([P, 1], f32)
                nc.vector.tensor_reduce(out=t_ered[:], in_=t_s2c[:], axis=mybir.AxisListType.X, op=Alu.add)
                nc.vector.tensor_tensor(out=accE[:], in0=accE[:], in1=t_ered[:], op=Alu.add)

            t_out = accp.tile([P, 4], f32)
            nc.vector.memset(t_out[:], 0.0)
            nc.vector.tensor_copy(out=t_out[:, 0:1], in_=accE[:])
            nc.vector.tensor_copy(out=t_out[:, 1:2], in_=accF[:])
            nc.vector.tensor_copy(out=t_out[:, 2:3], in_=accI[:])
            nc.sync.dma_start(out=d_out[:], in_=t_out[:])

    nc.compile()
    return nc


def kernel(u, v, c, prev_c, connectivities, N, dNdx, B, volumes):
    global LAST_EXEC_NS
    if "nc" not in _CACHE:
        _CACHE["nc"] = _build_bass()
    nc = _CACHE["nc"]
    from concourse.bass_utils import run_bass_kernel_spmd

    u = np.asarray(u, dtype=np.float32)
    v = np.asarray(v, dtype=np.float32)
    c = np.asarray(c, dtype=np.float32)
    prev_c = np.asarray(prev_c, dtype=np.float32)
    conn = np.asarray(connectivities)
    bf = ml_dtypes.bfloat16

    # node -> element-space layout prep (uvc triples per element corner)
    uvc_full = np.stack([u, v, c], axis=1)                 # [N_NODES, 3] f32
    uvc_el = uvc_full[conn.reshape(-1)].reshape(N_ELEMS, 12)
    n_bf = np.ascontiguousarray(N, dtype=np.float32).astype(bf).reshape(N_ELEMS, 16)
    dn_bf = np.ascontiguousarray(dNdx, dtype=np.float32).astype(bf).reshape(N_ELEMS, 32)
    b_bf = np.ascontiguousarray(B, dtype=np.float32).astype(bf).reshape(N_ELEMS, 96)
    vol = np.ascontiguousarray(volumes, dtype=np.float32).reshape(N_ELEMS, 4)

    c_pad = np.zeros(NODE_PAD * NCORES, np.float32)
    c_pad[:N_NODES] = c
    pc_pad = np.zeros(NODE_PAD * NCORES, np.float32)
    pc_pad[:N_NODES] = prev_c

    in_maps = []
    for i in range(NCORES):
        es = slice(i * EC, (i + 1) * EC)
        ns = slice(i * NODE_PAD, (i + 1) * NODE_PAD)
        in_maps.append({
            "uvc": uvc_el[es].reshape(P, EPP * 12),
            "nmat": n_bf[es].reshape(P, EPP * 16),
            "dmat": dn_bf[es].reshape(P, EPP * 32),
            "bmat": b_bf[es].reshape(P, EPP * 96),
            "vol": vol[es].reshape(P, EPP * 4),
            "cnd": c_pad[ns].reshape(P, NODE_F),
            "pnd": pc_pad[ns].reshape(P, NODE_F),
        })

    r = run_bass_kernel_spmd(nc, in_maps, core_ids=list(range(NCORES)), trace=TRACE)
    LAST_EXEC_NS = r.exec_time_ns

    parts = np.stack([np.asarray(r.results[i]["out"], dtype=np.float64) for i in range(NCORES)])
    sums = parts.sum(axis=(0, 1))                          # [4]
    e_el = sums[0]
    e_fr = (G_C / (2.0 * L_0)) * sums[1]
    e_ir = 0.5 * PENALTY * sums[2]
    return (np.float32(e_el), np.float32(e_fr), np.float32(e_ir))

